# revision 5
# baseline (speedup 1.0000x reference)
"""Trainium2 Bass kernel for nn_AutoGraderPrototypeModel (retrieval_knn).

Computes, for full inputs hidden_states [1024, 256, 1024] f32 and
prototype_weight [512, 1024] f32:

    a      = mean(hidden_states, axis=1)                  # [B, D]
    logits = 2 a @ proto.T - ||a||^2 - ||proto||^2        # [B, 512]
    out    = logits.reshape(B, 64, 8).mean(axis=1)        # [B, 8]

Sharding: data-parallel over batch across 8 NeuronCores (128 batch rows
per core, prototype table replicated). The dominant cost is streaming the
128 MiB hidden_states shard from HBM; pooling accumulates on the vector
engine under the DMA, and the distance computation is a small PSUM-
accumulated matmul tail.
"""

from contextlib import ExitStack

import numpy as np

B, T, D = 1024, 256, 1024
M_PROTO = 512
NUM_LABELS = 8
NUM_PROTOTYPES = 64
N_CORES = 8
BS = B // N_CORES  # 128 batch rows per core
P = 128            # SBUF partitions
T_CHUNK = 8        # t-steps per DMA tile -> [128, 8, 1024] f32 = 4 MiB

_cached_nc = None


def _build_program():
    import concourse.bass as bass  # noqa: F401
    import concourse.mybir as mybir
    import concourse.tile as tile
    from concourse import bacc, masks

    f32 = mybir.dt.float32
    KD = D // P            # 8 contraction chunks of 128 over D
    NT = T // T_CHUNK      # 32 DMA tiles over time
    MG = M_PROTO // P      # 4 prototype groups of 128

    nc = bacc.Bacc("TRN2", target_bir_lowering=False, debug=False,
                   num_devices=N_CORES)
    hs = nc.dram_tensor("hidden_states", [BS, T, D], f32, kind="ExternalInput").ap()
    pw = nc.dram_tensor("prototype_weight", [M_PROTO, D], f32, kind="ExternalInput").ap()
    out = nc.dram_tensor("out", [BS, NUM_LABELS], f32, kind="ExternalOutput").ap()

    with tile.TileContext(nc) as tc, ExitStack() as ctx:
        hs_pool = ctx.enter_context(tc.tile_pool(name="hs", bufs=3))
        work = ctx.enter_context(tc.tile_pool(name="work", bufs=1))
        psum_t = ctx.enter_context(tc.tile_pool(name="psum_t", bufs=2, space="PSUM"))
        psum_a = ctx.enter_context(tc.tile_pool(name="psum_a", bufs=1, space="PSUM"))

        ident = work.tile([P, P], f32, tag="ident")
        masks.make_identity(nc, ident[:])
        ones_m1 = work.tile([P, 1], f32, tag="ones_m1")   # lhsT K=128, M=1
        nc.gpsimd.memset(ones_m1[:], 1.0)
        ones_k1 = work.tile([1, P], f32, tag="ones_k1")   # lhsT K=1, M=128
        nc.gpsimd.memset(ones_k1[:], 1.0)

        # --- prototype prep: protoT2[k] = 2 * proto.T d-chunk, sqT[k] = (proto.T)^2
        proto_sb = []
        for j in range(MG):
            pj = work.tile([P, D], f32, tag=f"proto{j}")
            nc.sync.dma_start(pj[:], pw[j * P:(j + 1) * P, :])
            proto_sb.append(pj)

        protoT2 = [work.tile([P, M_PROTO], f32, tag=f"pT2_{k}", name=f"pT2_{k}")
                   for k in range(KD)]
        sqT = [work.tile([P, M_PROTO], f32, tag=f"sqT_{k}", name=f"sqT_{k}")
               for k in range(KD)]
        for k in range(KD):
            for j in range(MG):
                pt = psum_t.tile([P, P], f32, tag="tp")
                nc.tensor.transpose(pt[:], proto_sb[j][:, k * P:(k + 1) * P], ident[:])
                nc.scalar.mul(protoT2[k][:, j * P:(j + 1) * P], pt[:], 2.0)
            # (2 protoT)^2 = 4 protoT^2; compensated below via -0.25 scale
            nc.vector.tensor_mul(sqT[k][:], protoT2[k][:], protoT2[k][:])

        # b_sq[m] = sum_d proto[m,d]^2, as a [1, 512] row via ones-matmul
        bsq_ps = psum_a.tile([1, M_PROTO], f32, tag="bsq")
        for k in range(KD):
            nc.tensor.matmul(bsq_ps[:], ones_m1[:], sqT[k][:],
                             start=(k == 0), stop=(k == KD - 1))
        neg_bsq = work.tile([1, M_PROTO], f32, tag="neg_bsq")
        nc.scalar.mul(neg_bsq[:], bsq_ps[:], -0.25)

        # --- mean pool over time: acc[b, d] = sum_t hs[b, t, d]
        acc = work.tile([P, D], f32, tag="acc")
        dma_eng = [nc.sync, nc.scalar]
        for it in range(NT):
            tl = hs_pool.tile([P, T_CHUNK, D], f32, tag="hs")
            dma_eng[it % 2].dma_start(tl[:], hs[:, it * T_CHUNK:(it + 1) * T_CHUNK, :])
            for j in range(T_CHUNK):
                if it == 0 and j == 0:
                    nc.vector.tensor_add(acc[:], tl[:, 0, :], tl[:, 1, :])
                elif it == 0 and j == 1:
                    continue
                else:
                    nc.vector.tensor_add(acc[:], acc[:], tl[:, j, :])

        a_sb = work.tile([P, D], f32, tag="a")
        nc.scalar.mul(a_sb[:], acc[:], 1.0 / T)

        # a_sq[b] = sum_d a^2 as per-partition scalar [128, 1]
        sq_tmp = work.tile([P, D], f32, tag="sq_tmp")
        asq = work.tile([P, 1], f32, tag="asq")
        nc.vector.tensor_mul(sq_tmp[:], a_sb[:], a_sb[:])
        nc.vector.tensor_reduce(asq[:], sq_tmp[:], axis=mybir.AxisListType.X,
                                op=mybir.AluOpType.add)

        # aT[k] = a.T d-chunk [128d, 128b]
        aTs = []
        for k in range(KD):
            pt = psum_t.tile([P, P], f32, tag="tp")
            nc.tensor.transpose(pt[:], a_sb[:, k * P:(k + 1) * P], ident[:])
            aT = work.tile([P, P], f32, tag=f"aT{k}")
            nc.vector.tensor_copy(aT[:], pt[:])
            aTs.append(aT)

        # logits_pre[b, m] = 2 a@proto.T - b_sq, accumulated in one PSUM bank
        lg_ps = psum_a.tile([P, M_PROTO], f32, tag="lg")
        for k in range(KD):
            nc.tensor.matmul(lg_ps[:], aTs[k][:], protoT2[k][:],
                             start=(k == 0), stop=False)
        nc.tensor.matmul(lg_ps[:], ones_k1[:], neg_bsq[:], start=False, stop=True)

        # subtract a_sq (per-partition scalar broadcast along free dim)
        lg_sb = work.tile([P, M_PROTO], f32, tag="lg_sb")
        nc.vector.tensor_scalar_sub(lg_sb[:], lg_ps[:], asq[:])

        # label mean: out[b, l] = mean_p logits_pre[b, p*8 + l]
        out_sb = work.tile([P, NUM_LABELS], f32, tag="out_sb")
        lgv = lg_sb[:].rearrange("b (p l) -> b l p", l=NUM_LABELS)
        nc.vector.tensor_reduce(out_sb[:], lgv, axis=mybir.AxisListType.X,
                                op=mybir.AluOpType.add)
        nc.scalar.mul(out_sb[:], out_sb[:], 1.0 / NUM_PROTOTYPES)
        nc.sync.dma_start(out[:, :], out_sb[:])

    nc.compile()
    return nc


def _get_program():
    global _cached_nc
    if _cached_nc is None:
        _cached_nc = _build_program()
    return _cached_nc


def _make_in_maps(hs, pw):
    return [
        {
            "hidden_states": np.ascontiguousarray(hs[i * BS:(i + 1) * BS]),
            "prototype_weight": pw,
        }
        for i in range(N_CORES)
    ]


def run(hidden_states, prototype_weight, trace=False):
    """Run the SPMD kernel; returns (full_output, BassKernelResults)."""
    from concourse.bass_utils import run_bass_kernel_spmd

    hs = np.ascontiguousarray(np.asarray(hidden_states, dtype=np.float32))
    pw = np.ascontiguousarray(np.asarray(prototype_weight, dtype=np.float32))
    assert hs.shape == (B, T, D), hs.shape
    assert pw.shape == (M_PROTO, D), pw.shape

    nc = _get_program()
    res = run_bass_kernel_spmd(nc, _make_in_maps(hs, pw),
                               core_ids=list(range(N_CORES)), trace=trace)
    full = np.concatenate([res.results[i]["out"] for i in range(N_CORES)], axis=0)
    return full, res


def kernel(hidden_states, prototype_weight):
    full, _ = run(hidden_states, prototype_weight, trace=False)
    return full


# revision 7
# speedup vs baseline: 131.1369x; 131.1369x over previous
"""Trainium2 Bass kernel for nn_AutoGraderPrototypeModel (retrieval_knn).

Computes, for full inputs hidden_states [1024, 256, 1024] f32 and
prototype_weight [512, 1024] f32:

    a      = mean(hidden_states, axis=1)                  # [B, D]
    logits = 2 a @ proto.T - ||a||^2 - ||proto||^2        # [B, 512]
    out    = logits.reshape(B, 64, 8).mean(axis=1)        # [B, 8]

Sharding: data-parallel over batch across 8 NeuronCores (128 batch rows
per core, prototype table replicated). The dominant cost is streaming the
128 MiB hidden_states shard from HBM; pooling accumulates on the vector
engine under the DMA, and the distance computation is a small PSUM-
accumulated matmul tail.
"""

from contextlib import ExitStack

import numpy as np

B, T, D = 1024, 256, 1024
M_PROTO = 512
NUM_LABELS = 8
NUM_PROTOTYPES = 64
N_CORES = 8
BS = B // N_CORES  # 128 batch rows per core
P = 128            # SBUF partitions
T_CHUNK = 8        # t-steps per DMA tile -> [128, 8, 1024] f32 = 4 MiB

_cached = {}


def _build_program(reps=1):
    import concourse.mybir as mybir
    import concourse.tile as tile
    from concourse import bacc, masks

    f32 = mybir.dt.float32
    KD = D // P            # 8 contraction chunks of 128 over D
    NT = T // T_CHUNK      # 32 DMA tiles over time
    MG = M_PROTO // P      # 4 prototype groups of 128

    nc = bacc.Bacc("TRN2", target_bir_lowering=False, debug=False,
                   num_devices=N_CORES)
    hs = nc.dram_tensor("hidden_states", [BS, T, D], f32, kind="ExternalInput").ap()
    pw = nc.dram_tensor("prototype_weight", [M_PROTO, D], f32, kind="ExternalInput").ap()
    out = nc.dram_tensor("out", [BS, NUM_LABELS], f32, kind="ExternalOutput").ap()

    with tile.TileContext(nc) as tc, ExitStack() as ctx:
        hs_pool = ctx.enter_context(tc.tile_pool(name="hs", bufs=3))
        work = ctx.enter_context(tc.tile_pool(name="work", bufs=1))
        psum_t = ctx.enter_context(tc.tile_pool(name="psum_t", bufs=2, space="PSUM"))
        psum_a = ctx.enter_context(tc.tile_pool(name="psum_a", bufs=1, space="PSUM"))

        def body():
            ident = work.tile([P, P], f32, tag="ident", name="ident")
            masks.make_identity(nc, ident[:])
            ones_m1 = work.tile([P, 1], f32, tag="ones_m1", name="ones_m1")
            nc.gpsimd.memset(ones_m1[:], 1.0)
            ones_k1 = work.tile([1, P], f32, tag="ones_k1", name="ones_k1")
            nc.gpsimd.memset(ones_k1[:], 1.0)

            # protoT2[k] = 2 * proto.T d-chunk; sqT[k] = (2 proto.T)^2
            proto_sb = []
            for j in range(MG):
                pj = work.tile([P, D], f32, tag=f"proto{j}", name=f"proto{j}")
                nc.sync.dma_start(pj[:], pw[j * P:(j + 1) * P, :])
                proto_sb.append(pj)

            protoT2 = [work.tile([P, M_PROTO], f32, tag=f"pT2_{k}", name=f"pT2_{k}")
                       for k in range(KD)]
            sqT = [work.tile([P, M_PROTO], f32, tag=f"sqT_{k}", name=f"sqT_{k}")
                   for k in range(KD)]
            for k in range(KD):
                for j in range(MG):
                    pt = psum_t.tile([P, P], f32, tag="tp", name="pt")
                    nc.tensor.transpose(pt[:], proto_sb[j][:, k * P:(k + 1) * P],
                                        ident[:])
                    nc.scalar.mul(protoT2[k][:, j * P:(j + 1) * P], pt[:], 2.0)
                # (2 protoT)^2 = 4 protoT^2; compensated below via -0.25 scale
                nc.vector.tensor_mul(sqT[k][:], protoT2[k][:], protoT2[k][:])

            # b_sq[m] as a [1, 512] row via ones-matmul over squared protoT
            bsq_ps = psum_a.tile([1, M_PROTO], f32, tag="bsq", name="bsq_ps")
            for k in range(KD):
                nc.tensor.matmul(bsq_ps[:], ones_m1[:], sqT[k][:],
                                 start=(k == 0), stop=(k == KD - 1))
            neg_bsq = work.tile([1, M_PROTO], f32, tag="neg_bsq", name="neg_bsq")
            nc.scalar.mul(neg_bsq[:], bsq_ps[:], -0.25)

            # --- mean pool over time: acc[b, d] = sum_t hs[b, t, d]
            acc = work.tile([P, D], f32, tag="acc", name="acc")
            dma_eng = [nc.sync, nc.scalar]
            for it in range(NT):
                tl = hs_pool.tile([P, T_CHUNK, D], f32, tag="hs", name="tl")
                dma_eng[it % 2].dma_start(
                    tl[:], hs[:, it * T_CHUNK:(it + 1) * T_CHUNK, :])
                for j in range(T_CHUNK):
                    if it == 0 and j == 0:
                        nc.vector.tensor_add(acc[:], tl[:, 0, :], tl[:, 1, :])
                    elif it == 0 and j == 1:
                        continue
                    else:
                        nc.vector.tensor_add(acc[:], acc[:], tl[:, j, :])

            a_sb = work.tile([P, D], f32, tag="a", name="a_sb")
            nc.scalar.mul(a_sb[:], acc[:], 1.0 / T)

            # a_sq[b] = sum_d a^2 as per-partition scalar [128, 1]
            sq_tmp = work.tile([P, D], f32, tag="sq_tmp", name="sq_tmp")
            asq = work.tile([P, 1], f32, tag="asq", name="asq")
            nc.vector.tensor_mul(sq_tmp[:], a_sb[:], a_sb[:])
            nc.vector.tensor_reduce(asq[:], sq_tmp[:], axis=mybir.AxisListType.X,
                                    op=mybir.AluOpType.add)

            # aT[k] = a.T d-chunk [128d, 128b]
            aTs = []
            for k in range(KD):
                pt = psum_t.tile([P, P], f32, tag="tp", name="pt")
                nc.tensor.transpose(pt[:], a_sb[:, k * P:(k + 1) * P], ident[:])
                aT = work.tile([P, P], f32, tag=f"aT{k}", name=f"aT{k}")
                nc.vector.tensor_copy(aT[:], pt[:])
                aTs.append(aT)

            # logits_pre[b, m] = 2 a@proto.T - b_sq in one PSUM bank
            lg_ps = psum_a.tile([P, M_PROTO], f32, tag="lg", name="lg_ps")
            for k in range(KD):
                nc.tensor.matmul(lg_ps[:], aTs[k][:], protoT2[k][:],
                                 start=(k == 0), stop=False)
            nc.tensor.matmul(lg_ps[:], ones_k1[:], neg_bsq[:],
                             start=False, stop=True)

            # subtract a_sq (per-partition scalar broadcast along free dim)
            lg_sb = work.tile([P, M_PROTO], f32, tag="lg_sb", name="lg_sb")
            nc.vector.tensor_scalar_sub(lg_sb[:], lg_ps[:], asq[:])

            # label mean: out[b, l] = mean_p logits_pre[b, p*8 + l]
            out_sb = work.tile([P, NUM_LABELS], f32, tag="out_sb", name="out_sb")
            lgv = lg_sb[:].rearrange("b (p l) -> b l p", l=NUM_LABELS)
            nc.vector.tensor_reduce(out_sb[:], lgv, axis=mybir.AxisListType.X,
                                    op=mybir.AluOpType.add)
            nc.scalar.mul(out_sb[:], out_sb[:], 1.0 / NUM_PROTOTYPES)
            nc.sync.dma_start(out[:, :], out_sb[:])

        if reps == 1:
            body()
        else:
            hints = (mybir.EngineType.DVE, mybir.EngineType.PE,
                     mybir.EngineType.Activation, mybir.EngineType.SP,
                     mybir.EngineType.Pool)
            with tc.For_i(0, reps, 1, hint_engines=hints):
                body()

    nc.compile()
    return nc


def _get_program(reps=1):
    if reps not in _cached:
        _cached[reps] = _build_program(reps)
    return _cached[reps]


def _make_in_maps(hs, pw):
    return [
        {
            "hidden_states": np.ascontiguousarray(hs[i * BS:(i + 1) * BS]),
            "prototype_weight": pw,
        }
        for i in range(N_CORES)
    ]


def run(hidden_states, prototype_weight, trace=False, reps=1):
    """Run the SPMD kernel; returns (full_output, BassKernelResults)."""
    from concourse.bass_utils import run_bass_kernel_spmd

    hs = np.ascontiguousarray(np.asarray(hidden_states, dtype=np.float32))
    pw = np.ascontiguousarray(np.asarray(prototype_weight, dtype=np.float32))
    assert hs.shape == (B, T, D), hs.shape
    assert pw.shape == (M_PROTO, D), pw.shape

    nc = _get_program(reps)
    res = run_bass_kernel_spmd(nc, _make_in_maps(hs, pw),
                               core_ids=list(range(N_CORES)), trace=trace)
    full = np.concatenate([res.results[i]["out"] for i in range(N_CORES)], axis=0)
    return full, res


def kernel(hidden_states, prototype_weight):
    full, _ = run(hidden_states, prototype_weight, trace=False)
    return full


# revision 8
# speedup vs baseline: 223.4022x; 1.7036x over previous
"""Trainium2 Bass kernel for nn_AutoGraderPrototypeModel (retrieval_knn).

Computes, for full inputs hidden_states [1024, 256, 1024] f32 and
prototype_weight [512, 1024] f32:

    a      = mean(hidden_states, axis=1)                  # [B, D]
    logits = 2 a @ proto.T - ||a||^2 - ||proto||^2        # [B, 512]
    out    = logits.reshape(B, 64, 8).mean(axis=1)        # [B, 8]

Sharding: data-parallel over batch across 8 NeuronCores (128 batch rows
per core, prototype table replicated). The dominant cost is streaming the
128 MiB hidden_states shard from HBM.

DMA layout: strided partition reads (1 MiB partition stride) only reach
~190 GB/s/core on this part, while fully-linear reads reach ~340 GB/s.
So tiles are loaded as flat contiguous [128, 8192] blocks: partition p of
tile i holds 8 consecutive t-rows of batch b = 4i + p//32. Pooling then
runs in two stages: 7 in-partition DVE adds -> partial[p, d], then a PE
matmul with a sliding block-ones mask (value 1/T) accumulates the 32
partitions of each batch across all tiles into PSUM a[128b, 1024d].
"""

from contextlib import ExitStack

import numpy as np

B, T, D = 1024, 256, 1024
M_PROTO = 512
NUM_LABELS = 8
NUM_PROTOTYPES = 64
N_CORES = 8
BS = B // N_CORES  # 128 batch rows per core
P = 128            # SBUF partitions
T_CHUNK = 8        # t-rows per partition -> [128, 8192] f32 = 4 MiB per tile

_cached = {}


def _build_program(reps=1):
    import concourse.mybir as mybir
    import concourse.tile as tile
    from concourse import bacc, masks

    f32 = mybir.dt.float32
    KD = D // P                    # 8 contraction chunks of 128 over D
    NT = (BS * T) // (P * T_CHUNK)  # 32 linear tiles
    MG = M_PROTO // P              # 4 prototype groups of 128
    BPT = (P * T_CHUNK) // T       # 4 batches per tile

    nc = bacc.Bacc("TRN2", target_bir_lowering=False, debug=False,
                   num_devices=N_CORES)
    hs = nc.dram_tensor("hidden_states", [BS, T, D], f32, kind="ExternalInput").ap()
    pw = nc.dram_tensor("prototype_weight", [M_PROTO, D], f32, kind="ExternalInput").ap()
    out = nc.dram_tensor("out", [BS, NUM_LABELS], f32, kind="ExternalOutput").ap()

    hs_flat = hs.rearrange("b t d -> (b t d)")
    words_per_tile = P * T_CHUNK * D // 1  # elements per linear tile

    with tile.TileContext(nc) as tc, ExitStack() as ctx:
        hs_pool = ctx.enter_context(tc.tile_pool(name="hs", bufs=3))
        part_pool = ctx.enter_context(tc.tile_pool(name="part", bufs=3))
        work = ctx.enter_context(tc.tile_pool(name="work", bufs=1))
        psum_t = ctx.enter_context(tc.tile_pool(name="psum_t", bufs=2, space="PSUM"))
        psum_a = ctx.enter_context(tc.tile_pool(name="psum_a", bufs=1, space="PSUM"))

        def body():
            ident = work.tile([P, P], f32, tag="ident", name="ident")
            masks.make_identity(nc, ident[:])
            ones_m1 = work.tile([P, 1], f32, tag="ones_m1", name="ones_m1")
            nc.gpsimd.memset(ones_m1[:], 1.0)
            ones_k1 = work.tile([1, P], f32, tag="ones_k1", name="ones_k1")
            nc.gpsimd.memset(ones_k1[:], 1.0)

            # Sliding block-ones mask for stage-2 pooling:
            # zp[p, c] = 1/T iff c == 128 + p//32; lhsT_i = zp[:, 128-4i : 256-4i]
            # gives lhsT_i[p, m] = 1/T iff m == 4i + p//32.
            zp = work.tile([P, 2 * P], f32, tag="zp", name="zp")
            nc.gpsimd.memset(zp[:], 0.0)
            for c in range(BPT):
                nc.gpsimd.memset(zp[32 * c:32 * (c + 1), P + c:P + c + 1], 1.0 / T)

            # protoT2[k] = 2 * proto.T d-chunk; sqT[k] = (2 proto.T)^2
            proto_sb = []
            for j in range(MG):
                pj = work.tile([P, D], f32, tag=f"proto{j}", name=f"proto{j}")
                nc.sync.dma_start(pj[:], pw[j * P:(j + 1) * P, :])
                proto_sb.append(pj)

            protoT2 = [work.tile([P, M_PROTO], f32, tag=f"pT2_{k}", name=f"pT2_{k}")
                       for k in range(KD)]
            sqT = [work.tile([P, M_PROTO], f32, tag=f"sqT_{k}", name=f"sqT_{k}")
                   for k in range(KD)]
            for k in range(KD):
                for j in range(MG):
                    pt = psum_t.tile([P, P], f32, tag="tp", name="pt")
                    nc.tensor.transpose(pt[:], proto_sb[j][:, k * P:(k + 1) * P],
                                        ident[:])
                    nc.scalar.mul(protoT2[k][:, j * P:(j + 1) * P], pt[:], 2.0)
                # (2 protoT)^2 = 4 protoT^2; compensated below via -0.25 scale
                nc.vector.tensor_mul(sqT[k][:], protoT2[k][:], protoT2[k][:])

            # b_sq[m] as a [1, 512] row via ones-matmul over squared protoT
            bsq_ps = psum_a.tile([1, M_PROTO], f32, tag="bsq", name="bsq_ps")
            for k in range(KD):
                nc.tensor.matmul(bsq_ps[:], ones_m1[:], sqT[k][:],
                                 start=(k == 0), stop=(k == KD - 1))
            neg_bsq = work.tile([1, M_PROTO], f32, tag="neg_bsq", name="neg_bsq")
            nc.scalar.mul(neg_bsq[:], bsq_ps[:], -0.25)

            # --- pooling: a[b, d] = (1/T) sum_t hs[b, t, d]
            a_ps = psum_a.tile([P, D], f32, tag="a_ps", name="a_ps")
            dma_eng = [nc.sync, nc.scalar]
            for it in range(NT):
                tl = hs_pool.tile([P, T_CHUNK * D], f32, tag="hs", name="tl")
                src = hs_flat[it * words_per_tile:(it + 1) * words_per_tile]
                dma_eng[it % 2].dma_start(tl[:], src.rearrange("(p w) -> p w", p=P))
                # stage 1: reduce the 8 in-partition t-rows
                partial = part_pool.tile([P, D], f32, tag="part", name="partial")
                nc.vector.tensor_add(partial[:], tl[:, 0:D], tl[:, D:2 * D])
                for j in range(2, T_CHUNK):
                    nc.vector.tensor_add(partial[:], partial[:],
                                         tl[:, j * D:(j + 1) * D])
                # stage 2: cross-partition scatter-accumulate via PE
                lhsT = zp[:, P - BPT * it:2 * P - BPT * it]
                for h in range(2):
                    nc.tensor.matmul(a_ps[:, h * 512:(h + 1) * 512], lhsT,
                                     partial[:, h * 512:(h + 1) * 512],
                                     start=(it == 0), stop=(it == NT - 1),
                                     skip_group_check=True)

            a_sb = work.tile([P, D], f32, tag="a", name="a_sb")
            nc.scalar.mul(a_sb[:], a_ps[:], 1.0)

            # a_sq[b] = sum_d a^2 as per-partition scalar [128, 1]
            sq_tmp = work.tile([P, D], f32, tag="sq_tmp", name="sq_tmp")
            asq = work.tile([P, 1], f32, tag="asq", name="asq")
            nc.vector.tensor_mul(sq_tmp[:], a_sb[:], a_sb[:])
            nc.vector.tensor_reduce(asq[:], sq_tmp[:], axis=mybir.AxisListType.X,
                                    op=mybir.AluOpType.add)

            # aT[k] = a.T d-chunk [128d, 128b]
            aTs = []
            for k in range(KD):
                pt = psum_t.tile([P, P], f32, tag="tp", name="pt")
                nc.tensor.transpose(pt[:], a_sb[:, k * P:(k + 1) * P], ident[:])
                aT = work.tile([P, P], f32, tag=f"aT{k}", name=f"aT{k}")
                nc.vector.tensor_copy(aT[:], pt[:])
                aTs.append(aT)

            # logits_pre[b, m] = 2 a@proto.T - b_sq in one PSUM bank
            lg_ps = psum_a.tile([P, M_PROTO], f32, tag="lg", name="lg_ps")
            for k in range(KD):
                nc.tensor.matmul(lg_ps[:], aTs[k][:], protoT2[k][:],
                                 start=(k == 0), stop=False)
            nc.tensor.matmul(lg_ps[:], ones_k1[:], neg_bsq[:],
                             start=False, stop=True)

            # subtract a_sq (per-partition scalar broadcast along free dim)
            lg_sb = work.tile([P, M_PROTO], f32, tag="lg_sb", name="lg_sb")
            nc.vector.tensor_scalar_sub(lg_sb[:], lg_ps[:], asq[:])

            # label mean: out[b, l] = mean_p logits_pre[b, p*8 + l]
            out_sb = work.tile([P, NUM_LABELS], f32, tag="out_sb", name="out_sb")
            lgv = lg_sb[:].rearrange("b (p l) -> b l p", l=NUM_LABELS)
            nc.vector.tensor_reduce(out_sb[:], lgv, axis=mybir.AxisListType.X,
                                    op=mybir.AluOpType.add)
            nc.scalar.mul(out_sb[:], out_sb[:], 1.0 / NUM_PROTOTYPES)
            nc.sync.dma_start(out[:, :], out_sb[:])

        if reps == 1:
            body()
        else:
            hints = (mybir.EngineType.DVE, mybir.EngineType.PE,
                     mybir.EngineType.Activation, mybir.EngineType.SP,
                     mybir.EngineType.Pool)
            with tc.For_i(0, reps, 1, hint_engines=hints):
                body()

    nc.compile()
    return nc


def _get_program(reps=1):
    if reps not in _cached:
        _cached[reps] = _build_program(reps)
    return _cached[reps]


def _make_in_maps(hs, pw):
    return [
        {
            "hidden_states": np.ascontiguousarray(hs[i * BS:(i + 1) * BS]),
            "prototype_weight": pw,
        }
        for i in range(N_CORES)
    ]


def run(hidden_states, prototype_weight, trace=False, reps=1):
    """Run the SPMD kernel; returns (full_output, BassKernelResults)."""
    from concourse.bass_utils import run_bass_kernel_spmd

    hs = np.ascontiguousarray(np.asarray(hidden_states, dtype=np.float32))
    pw = np.ascontiguousarray(np.asarray(prototype_weight, dtype=np.float32))
    assert hs.shape == (B, T, D), hs.shape
    assert pw.shape == (M_PROTO, D), pw.shape

    nc = _get_program(reps)
    res = run_bass_kernel_spmd(nc, _make_in_maps(hs, pw),
                               core_ids=list(range(N_CORES)), trace=trace)
    full = np.concatenate([res.results[i]["out"] for i in range(N_CORES)], axis=0)
    return full, res


def kernel(hidden_states, prototype_weight):
    full, _ = run(hidden_states, prototype_weight, trace=False)
    return full


# revision 12
# speedup vs baseline: 242.1120x; 1.0837x over previous
"""Trainium2 Bass kernel for nn_AutoGraderPrototypeModel (retrieval_knn).

Computes, for full inputs hidden_states [1024, 256, 1024] f32 and
prototype_weight [512, 1024] f32:

    a      = mean(hidden_states, axis=1)                  # [B, D]
    logits = 2 a @ proto.T - ||a||^2 - ||proto||^2        # [B, 512]
    out    = logits.reshape(B, 64, 8).mean(axis=1)        # [B, 8]

Sharding: data-parallel over batch across 8 NeuronCores (128 batch rows
per core, prototype table replicated). The dominant cost is streaming the
128 MiB hidden_states shard from HBM.

DMA layout: strided partition reads (1 MiB partition stride) only reach
~190 GB/s/core on this part, while fully-linear reads reach ~350 GB/s.
Tiles are therefore loaded as flat contiguous [128, WPP] blocks. With
WPP words per partition, partition p of tile i holds WPP/1024 consecutive
t-rows; pooling reduces over t in up to two stages:
  stage 1 (only if WPP > 1024): DVE adds fold the in-partition t-rows;
  stage 2: a PE matmul with a sliding block-column mask (value 1/T)
  scatter-accumulates each batch's partitions into PSUM a[128b, 1024d].
At WPP=1024 the PE consumes raw tiles directly and the vector engine is
not involved in pooling at all.
"""

from contextlib import ExitStack

import numpy as np

B, T, D = 1024, 256, 1024
M_PROTO = 512
NUM_LABELS = 8
NUM_PROTOTYPES = 64
N_CORES = 8
BS = B // N_CORES  # 128 batch rows per core
P = 128            # SBUF partitions
WPP = 2048         # words per partition per DMA tile (tile = P*WPP*4 bytes)
HS_BUFS = 6

_cached = {}


def _build_program(reps=1, wpp=WPP, hs_bufs=HS_BUFS, act_pt2=False,
                   ttr_asq=False, psum_a_pool=True):
    import concourse.mybir as mybir
    import concourse.tile as tile
    from concourse import bacc, masks

    f32 = mybir.dt.float32
    KD = D // P                      # 8 contraction chunks of 128 over D
    MG = M_PROTO // P                # 4 prototype groups of 128
    words_per_tile = P * wpp
    NT = (BS * T * D) // words_per_tile  # linear tiles per shard
    n_rows = wpp // D                # t-rows per partition (stage-1 depth)
    assert wpp % D == 0
    # batches per tile as a fraction: bpt_num/bpt_den
    bpt_num, bpt_den = words_per_tile, T * D
    n_cols = max(bpt_num // bpt_den, 1)   # mask columns per tile
    grp = P // n_cols if bpt_num >= bpt_den else P

    nc = bacc.Bacc("TRN2", target_bir_lowering=False, debug=False,
                   num_devices=N_CORES)
    hs = nc.dram_tensor("hidden_states", [BS, T, D], f32, kind="ExternalInput").ap()
    pw = nc.dram_tensor("prototype_weight", [M_PROTO, D], f32, kind="ExternalInput").ap()
    out = nc.dram_tensor("out", [BS, NUM_LABELS], f32, kind="ExternalOutput").ap()

    hs_flat = hs.rearrange("b t d -> (b t d)")

    with tile.TileContext(nc) as tc, ExitStack() as ctx:
        hs_pool = ctx.enter_context(tc.tile_pool(name="hs", bufs=hs_bufs))
        part_pool = ctx.enter_context(tc.tile_pool(name="part", bufs=3))
        work = ctx.enter_context(tc.tile_pool(name="work", bufs=1))
        psum_t = ctx.enter_context(tc.tile_pool(name="psum_t", bufs=2, space="PSUM"))
        psum_a = ctx.enter_context(tc.tile_pool(name="psum_a", bufs=1, space="PSUM"))

        state = {}

        def prep():
            ident = work.tile([P, P], f32, tag="ident", name="ident")
            masks.make_identity(nc, ident[:])
            ones_m1 = work.tile([P, 1], f32, tag="ones_m1", name="ones_m1")
            nc.gpsimd.memset(ones_m1[:], 1.0)
            ones_k1 = work.tile([1, P], f32, tag="ones_k1", name="ones_k1")
            nc.gpsimd.memset(ones_k1[:], 1.0)

            # Sliding mask for stage-2 pooling: zp[p, P + c] = 1/T iff
            # c == p // grp (c < n_cols). lhsT for tile i is
            # zp[:, P - s_i : 2P - s_i] with s_i = floor(i * bpt).
            zp = work.tile([P, 2 * P], f32, tag="zp", name="zp")
            nc.gpsimd.memset(zp[:], 0.0)
            for c in range(n_cols):
                nc.gpsimd.memset(zp[grp * c:grp * (c + 1), P + c:P + c + 1],
                                 1.0 / T)

            # protoT2[k] = 2 * proto.T d-chunk; sqT[k] = (2 proto.T)^2
            proto_sb = []
            for j in range(MG):
                pj = work.tile([P, D], f32, tag=f"proto{j}", name=f"proto{j}")
                nc.gpsimd.dma_start(pj[:], pw[j * P:(j + 1) * P, :])
                proto_sb.append(pj)

            protoT2 = [work.tile([P, M_PROTO], f32, tag=f"pT2_{k}", name=f"pT2_{k}")
                       for k in range(KD)]
            sqT = [work.tile([P, M_PROTO], f32, tag=f"sqT_{k}", name=f"sqT_{k}")
                   for k in range(KD)]
            for k in range(KD):
                for j in range(MG):
                    pt = psum_t.tile([P, P], f32, tag="tp", name="pt")
                    nc.tensor.transpose(pt[:], proto_sb[j][:, k * P:(k + 1) * P],
                                        ident[:])
                    if act_pt2:
                        nc.scalar.mul(protoT2[k][:, j * P:(j + 1) * P],
                                      pt[:], 2.0)
                    else:
                        nc.vector.tensor_scalar_mul(
                            protoT2[k][:, j * P:(j + 1) * P], pt[:], 2.0)
                # (2 protoT)^2 = 4 protoT^2; compensated below via -0.25 scale
                nc.vector.tensor_mul(sqT[k][:], protoT2[k][:], protoT2[k][:])

            # b_sq[m] as a [1, 512] row via ones-matmul over squared protoT
            bsq_ps = psum_a.tile([1, M_PROTO], f32, tag="bsq", name="bsq_ps")
            for k in range(KD):
                nc.tensor.matmul(bsq_ps[:], ones_m1[:], sqT[k][:],
                                 start=(k == 0), stop=(k == KD - 1))
            neg_bsq = work.tile([1, M_PROTO], f32, tag="neg_bsq", name="neg_bsq")
            nc.scalar.mul(neg_bsq[:], bsq_ps[:], -0.25)

            state.update(ident=ident, ones_k1=ones_k1, zp=zp, neg_bsq=neg_bsq,
                         protoT2=protoT2)

        def stream():
            ident = state["ident"]
            zp = state["zp"]
            protoT2 = state["protoT2"]

            # --- pooling: a[b, d] = (1/T) sum_t hs[b, t, d], in PSUM
            a_ps = psum_a.tile([P, D], f32, tag="a_ps", name="a_ps")
            dma_eng = [nc.sync, nc.scalar]
            for it in range(NT):
                tl = hs_pool.tile([P, wpp], f32, tag="hs", name="tl")
                src = hs_flat[it * words_per_tile:(it + 1) * words_per_tile]
                dma_eng[it % 2].dma_start(tl[:], src.rearrange("(p w) -> p w", p=P))
                if n_rows > 1:
                    partial = part_pool.tile([P, D], f32, tag="part",
                                             name="partial")
                    nc.vector.tensor_add(partial[:], tl[:, 0:D], tl[:, D:2 * D])
                    for j in range(2, n_rows):
                        nc.vector.tensor_add(partial[:], partial[:],
                                             tl[:, j * D:(j + 1) * D])
                else:
                    partial = tl
                s_i = (it * bpt_num) // bpt_den
                lhsT = zp[:, P - s_i:2 * P - s_i]
                for h in range(2):
                    nc.tensor.matmul(a_ps[:, h * 512:(h + 1) * 512], lhsT,
                                     partial[:, h * 512:(h + 1) * 512],
                                     start=(it == 0), stop=(it == NT - 1),
                                     skip_group_check=True)

            a_sb = work.tile([P, D], f32, tag="a", name="a_sb")
            nc.scalar.mul(a_sb[:], a_ps[:], 1.0)

            # a_sq[b] = sum_d a^2 as per-partition scalar [128, 1]
            sq_tmp = work.tile([P, D], f32, tag="sq_tmp", name="sq_tmp")
            asq = work.tile([P, 1], f32, tag="asq", name="asq")
            if ttr_asq:
                nc.vector.tensor_tensor_reduce(
                    out=sq_tmp[:], in0=a_sb[:], in1=a_sb[:], scale=1.0,
                    scalar=0.0, op0=mybir.AluOpType.mult,
                    op1=mybir.AluOpType.add, accum_out=asq[:])
            else:
                nc.vector.tensor_mul(sq_tmp[:], a_sb[:], a_sb[:])
                nc.vector.tensor_reduce(asq[:], sq_tmp[:],
                                        axis=mybir.AxisListType.X,
                                        op=mybir.AluOpType.add)

            # aT[k] = a.T d-chunk [128d, 128b]
            aTs = []
            for k in range(KD):
                pt = psum_t.tile([P, P], f32, tag="tp", name="pt")
                nc.tensor.transpose(pt[:], a_sb[:, k * P:(k + 1) * P], ident[:])
                aT = work.tile([P, P], f32, tag=f"aT{k}", name=f"aT{k}")
                nc.vector.tensor_copy(aT[:], pt[:])
                aTs.append(aT)

            # logits_pre[b, m] = 2 a@proto.T - b_sq in one PSUM bank
            lg_ps = psum_a.tile([P, M_PROTO], f32, tag="lg", name="lg_ps")
            for k in range(KD):
                nc.tensor.matmul(lg_ps[:], aTs[k][:], protoT2[k][:],
                                 start=(k == 0), stop=False)
            nc.tensor.matmul(lg_ps[:], state["ones_k1"][:], state["neg_bsq"][:],
                             start=False, stop=True)

            # subtract a_sq (per-partition scalar broadcast along free dim)
            lg_sb = work.tile([P, M_PROTO], f32, tag="lg_sb", name="lg_sb")
            nc.vector.tensor_scalar_sub(lg_sb[:], lg_ps[:], asq[:])

            # label mean: out[b, l] = mean_p logits_pre[b, p*8 + l]
            out_sb = work.tile([P, NUM_LABELS], f32, tag="out_sb", name="out_sb")
            lgv = lg_sb[:].rearrange("b (p l) -> b l p", l=NUM_LABELS)
            nc.vector.tensor_reduce(out_sb[:], lgv, axis=mybir.AxisListType.X,
                                    op=mybir.AluOpType.add)
            nc.scalar.mul(out_sb[:], out_sb[:], 1.0 / NUM_PROTOTYPES)
            nc.gpsimd.dma_start(out[:, :], out_sb[:])

        prep()
        if reps == 1:
            stream()
        else:
            hints = (mybir.EngineType.DVE, mybir.EngineType.PE,
                     mybir.EngineType.Activation, mybir.EngineType.SP,
                     mybir.EngineType.Pool)
            with tc.For_i(0, reps, 1, hint_engines=hints):
                stream()

    nc.compile()
    return nc


def _get_program(reps=1, **kw):
    key = (reps, tuple(sorted(kw.items())))
    if key not in _cached:
        _cached[key] = _build_program(reps, **kw)
    return _cached[key]


def _make_in_maps(hs, pw):
    return [
        {
            "hidden_states": np.ascontiguousarray(hs[i * BS:(i + 1) * BS]),
            "prototype_weight": pw,
        }
        for i in range(N_CORES)
    ]


def run(hidden_states, prototype_weight, trace=False, reps=1):
    """Run the SPMD kernel; returns (full_output, BassKernelResults)."""
    from concourse.bass_utils import run_bass_kernel_spmd

    hs = np.ascontiguousarray(np.asarray(hidden_states, dtype=np.float32))
    pw = np.ascontiguousarray(np.asarray(prototype_weight, dtype=np.float32))
    assert hs.shape == (B, T, D), hs.shape
    assert pw.shape == (M_PROTO, D), pw.shape

    nc = _get_program(reps)
    res = run_bass_kernel_spmd(nc, _make_in_maps(hs, pw),
                               core_ids=list(range(N_CORES)), trace=trace)
    full = np.concatenate([res.results[i]["out"] for i in range(N_CORES)], axis=0)
    return full, res


def kernel(hidden_states, prototype_weight):
    full, _ = run(hidden_states, prototype_weight, trace=False)
    return full


# revision 14
# speedup vs baseline: 243.1214x; 1.0042x over previous
"""Trainium2 Bass kernel for nn_AutoGraderPrototypeModel (retrieval_knn).

Computes, for full inputs hidden_states [1024, 256, 1024] f32 and
prototype_weight [512, 1024] f32:

    a      = mean(hidden_states, axis=1)                  # [B, D]
    logits = 2 a @ proto.T - ||a||^2 - ||proto||^2        # [B, 512]
    out    = logits.reshape(B, 64, 8).mean(axis=1)        # [B, 8]

Sharding: data-parallel over batch across 8 NeuronCores (128 batch rows
per core, prototype table replicated). The dominant cost is streaming the
128 MiB hidden_states shard from HBM.

DMA layout: strided partition reads (1 MiB partition stride) only reach
~190 GB/s/core on this part, while fully-linear reads reach ~350 GB/s.
Tiles are therefore loaded as flat contiguous [128, WPP] blocks. With
WPP words per partition, partition p of tile i holds WPP/1024 consecutive
t-rows; pooling reduces over t in up to two stages:
  stage 1 (only if WPP > 1024): DVE adds fold the in-partition t-rows;
  stage 2: a PE matmul with a sliding block-column mask (value 1/T)
  scatter-accumulates each batch's partitions into PSUM a[128b, 1024d].
At WPP=1024 the PE consumes raw tiles directly and the vector engine is
not involved in pooling at all.
"""

import os

os.environ.setdefault("JAX_PLATFORMS", "axon,cpu")

from contextlib import ExitStack

import numpy as np

B, T, D = 1024, 256, 1024
M_PROTO = 512
NUM_LABELS = 8
NUM_PROTOTYPES = 64
N_CORES = 8
BS = B // N_CORES  # 128 batch rows per core
P = 128            # SBUF partitions
WPP = 2048         # words per partition per DMA tile (tile = P*WPP*4 bytes)
HS_BUFS = 6

_cached = {}


def _build_program(reps=1, wpp=WPP, hs_bufs=HS_BUFS, act_pt2=False,
                   ttr_asq=False, stage1=True):
    import concourse.mybir as mybir
    import concourse.tile as tile
    from concourse import bacc, masks

    f32 = mybir.dt.float32
    KD = D // P                      # 8 contraction chunks of 128 over D
    MG = M_PROTO // P                # 4 prototype groups of 128
    words_per_tile = P * wpp
    NT = (BS * T * D) // words_per_tile  # linear tiles per shard
    n_rows = wpp // D                # t-rows per partition (stage-1 depth)
    assert wpp % D == 0
    # batches per tile as a fraction: bpt_num/bpt_den
    bpt_num, bpt_den = words_per_tile, T * D
    n_cols = max(bpt_num // bpt_den, 1)   # mask columns per tile
    grp = P // n_cols if bpt_num >= bpt_den else P

    nc = bacc.Bacc("TRN2", target_bir_lowering=False, debug=False,
                   num_devices=N_CORES)
    hs = nc.dram_tensor("hidden_states", [BS, T, D], f32, kind="ExternalInput").ap()
    pw = nc.dram_tensor("prototype_weight", [M_PROTO, D], f32, kind="ExternalInput").ap()
    out = nc.dram_tensor("out", [BS, NUM_LABELS], f32, kind="ExternalOutput").ap()

    hs_flat = hs.rearrange("b t d -> (b t d)")

    with tile.TileContext(nc) as tc, ExitStack() as ctx:
        hs_pool = ctx.enter_context(tc.tile_pool(name="hs", bufs=hs_bufs))
        part_pool = ctx.enter_context(tc.tile_pool(name="part", bufs=3))
        work = ctx.enter_context(tc.tile_pool(name="work", bufs=1))
        psum_t = ctx.enter_context(tc.tile_pool(name="psum_t", bufs=2, space="PSUM"))
        psum_a = ctx.enter_context(tc.tile_pool(name="psum_a", bufs=1, space="PSUM"))

        state = {}

        def prep():
            ident = work.tile([P, P], f32, tag="ident", name="ident")
            masks.make_identity(nc, ident[:])
            ones_m1 = work.tile([P, 1], f32, tag="ones_m1", name="ones_m1")
            nc.gpsimd.memset(ones_m1[:], 1.0)
            ones_k1 = work.tile([1, P], f32, tag="ones_k1", name="ones_k1")
            nc.gpsimd.memset(ones_k1[:], 1.0)

            # Sliding mask for stage-2 pooling: zp[p, P + c] = 1/T iff
            # c == p // grp (c < n_cols). lhsT for tile i is
            # zp[:, P - s_i : 2P - s_i] with s_i = floor(i * bpt).
            zp = work.tile([P, 2 * P], f32, tag="zp", name="zp")
            nc.gpsimd.memset(zp[:], 0.0)
            for c in range(n_cols):
                nc.gpsimd.memset(zp[grp * c:grp * (c + 1), P + c:P + c + 1],
                                 1.0 / T)

            # protoT2[k] = 2 * proto.T d-chunk; sqT[k] = (2 proto.T)^2
            proto_sb = []
            for j in range(MG):
                pj = work.tile([P, D], f32, tag=f"proto{j}", name=f"proto{j}")
                nc.gpsimd.dma_start(pj[:], pw[j * P:(j + 1) * P, :])
                proto_sb.append(pj)

            protoT2 = [work.tile([P, M_PROTO], f32, tag=f"pT2_{k}", name=f"pT2_{k}")
                       for k in range(KD)]
            sqT = [work.tile([P, M_PROTO], f32, tag=f"sqT_{k}", name=f"sqT_{k}")
                   for k in range(KD)]
            for k in range(KD):
                for j in range(MG):
                    pt = psum_t.tile([P, P], f32, tag="tp", name="pt")
                    nc.tensor.transpose(pt[:], proto_sb[j][:, k * P:(k + 1) * P],
                                        ident[:])
                    if act_pt2:
                        nc.scalar.mul(protoT2[k][:, j * P:(j + 1) * P],
                                      pt[:], 2.0)
                    else:
                        nc.vector.tensor_scalar_mul(
                            protoT2[k][:, j * P:(j + 1) * P], pt[:], 2.0)
                # (2 protoT)^2 = 4 protoT^2; compensated below via -0.25 scale
                nc.vector.tensor_mul(sqT[k][:], protoT2[k][:], protoT2[k][:])

            # b_sq[m] as a [1, 512] row via ones-matmul over squared protoT
            bsq_ps = psum_a.tile([1, M_PROTO], f32, tag="bsq", name="bsq_ps")
            for k in range(KD):
                nc.tensor.matmul(bsq_ps[:], ones_m1[:], sqT[k][:],
                                 start=(k == 0), stop=(k == KD - 1))
            neg_bsq = work.tile([1, M_PROTO], f32, tag="neg_bsq", name="neg_bsq")
            nc.scalar.mul(neg_bsq[:], bsq_ps[:], -0.25)

            state.update(ident=ident, ones_k1=ones_k1, zp=zp, neg_bsq=neg_bsq,
                         protoT2=protoT2)

        def stream():
            ident = state["ident"]
            zp = state["zp"]
            protoT2 = state["protoT2"]

            # --- pooling: a[b, d] = (1/T) sum_t hs[b, t, d], in PSUM
            a_ps = psum_a.tile([P, D], f32, tag="a_ps", name="a_ps")
            dma_eng = [nc.sync, nc.scalar]
            for it in range(NT):
                tl = hs_pool.tile([P, wpp], f32, tag="hs", name="tl")
                src = hs_flat[it * words_per_tile:(it + 1) * words_per_tile]
                dma_eng[it % 2].dma_start(tl[:], src.rearrange("(p w) -> p w", p=P))
                s_i = (it * bpt_num) // bpt_den
                lhsT = zp[:, P - s_i:2 * P - s_i]
                if stage1 and n_rows > 1:
                    partial = part_pool.tile([P, D], f32, tag="part",
                                             name="partial")
                    nc.vector.tensor_add(partial[:], tl[:, 0:D], tl[:, D:2 * D])
                    for j in range(2, n_rows):
                        nc.vector.tensor_add(partial[:], partial[:],
                                             tl[:, j * D:(j + 1) * D])
                    for h in range(2):
                        nc.tensor.matmul(a_ps[:, h * 512:(h + 1) * 512], lhsT,
                                         partial[:, h * 512:(h + 1) * 512],
                                         start=(it == 0), stop=(it == NT - 1),
                                         skip_group_check=True)
                else:
                    # PE consumes raw t-rows directly; all rows of a tile
                    # share the same mask column (same batch coverage)
                    for r in range(n_rows):
                        for h in range(2):
                            nc.tensor.matmul(
                                a_ps[:, h * 512:(h + 1) * 512], lhsT,
                                tl[:, r * D + h * 512:r * D + (h + 1) * 512],
                                start=(it == 0 and r == 0),
                                stop=(it == NT - 1 and r == n_rows - 1),
                                skip_group_check=True)

            a_sb = work.tile([P, D], f32, tag="a", name="a_sb")
            nc.scalar.mul(a_sb[:], a_ps[:], 1.0)

            # a_sq[b] = sum_d a^2 as per-partition scalar [128, 1]
            sq_tmp = work.tile([P, D], f32, tag="sq_tmp", name="sq_tmp")
            asq = work.tile([P, 1], f32, tag="asq", name="asq")
            if ttr_asq:
                nc.vector.tensor_tensor_reduce(
                    out=sq_tmp[:], in0=a_sb[:], in1=a_sb[:], scale=1.0,
                    scalar=0.0, op0=mybir.AluOpType.mult,
                    op1=mybir.AluOpType.add, accum_out=asq[:])
            else:
                nc.vector.tensor_mul(sq_tmp[:], a_sb[:], a_sb[:])
                nc.vector.tensor_reduce(asq[:], sq_tmp[:],
                                        axis=mybir.AxisListType.X,
                                        op=mybir.AluOpType.add)

            # aT[k] = a.T d-chunk [128d, 128b]
            aTs = []
            for k in range(KD):
                pt = psum_t.tile([P, P], f32, tag="tp", name="pt")
                nc.tensor.transpose(pt[:], a_sb[:, k * P:(k + 1) * P], ident[:])
                aT = work.tile([P, P], f32, tag=f"aT{k}", name=f"aT{k}")
                nc.vector.tensor_copy(aT[:], pt[:])
                aTs.append(aT)

            # logits_pre[b, m] = 2 a@proto.T - b_sq in one PSUM bank
            lg_ps = psum_a.tile([P, M_PROTO], f32, tag="lg", name="lg_ps")
            for k in range(KD):
                nc.tensor.matmul(lg_ps[:], aTs[k][:], protoT2[k][:],
                                 start=(k == 0), stop=False)
            nc.tensor.matmul(lg_ps[:], state["ones_k1"][:], state["neg_bsq"][:],
                             start=False, stop=True)

            # subtract a_sq (per-partition scalar broadcast along free dim)
            lg_sb = work.tile([P, M_PROTO], f32, tag="lg_sb", name="lg_sb")
            nc.vector.tensor_scalar_sub(lg_sb[:], lg_ps[:], asq[:])

            # label mean: out[b, l] = mean_p logits_pre[b, p*8 + l]
            out_sb = work.tile([P, NUM_LABELS], f32, tag="out_sb", name="out_sb")
            lgv = lg_sb[:].rearrange("b (p l) -> b l p", l=NUM_LABELS)
            nc.vector.tensor_reduce(out_sb[:], lgv, axis=mybir.AxisListType.X,
                                    op=mybir.AluOpType.add)
            nc.scalar.mul(out_sb[:], out_sb[:], 1.0 / NUM_PROTOTYPES)
            nc.gpsimd.dma_start(out[:, :], out_sb[:])

        prep()
        if reps == 1:
            stream()
        else:
            hints = (mybir.EngineType.DVE, mybir.EngineType.PE,
                     mybir.EngineType.Activation, mybir.EngineType.SP,
                     mybir.EngineType.Pool)
            with tc.For_i(0, reps, 1, hint_engines=hints):
                stream()

    nc.compile()
    return nc


def _get_program(reps=1, **kw):
    key = (reps, tuple(sorted(kw.items())))
    if key not in _cached:
        _cached[key] = _build_program(reps, **kw)
    return _cached[key]


def _make_in_maps(hs, pw):
    return [
        {
            "hidden_states": np.ascontiguousarray(hs[i * BS:(i + 1) * BS]),
            "prototype_weight": pw,
        }
        for i in range(N_CORES)
    ]


def run(hidden_states, prototype_weight, trace=False, reps=1):
    """Run the SPMD kernel; returns (full_output, BassKernelResults)."""
    from concourse.bass_utils import run_bass_kernel_spmd

    hs = np.ascontiguousarray(np.asarray(hidden_states, dtype=np.float32))
    pw = np.ascontiguousarray(np.asarray(prototype_weight, dtype=np.float32))
    assert hs.shape == (B, T, D), hs.shape
    assert pw.shape == (M_PROTO, D), pw.shape

    nc = _get_program(reps)
    res = run_bass_kernel_spmd(nc, _make_in_maps(hs, pw),
                               core_ids=list(range(N_CORES)), trace=trace)
    full = np.concatenate([res.results[i]["out"] for i in range(N_CORES)], axis=0)
    return full, res


def kernel(hidden_states, prototype_weight):
    full, _ = run(hidden_states, prototype_weight, trace=False)
    return full


# revision 15
# speedup vs baseline: 244.7471x; 1.0067x over previous
"""Trainium2 Bass kernel for nn_AutoGraderPrototypeModel (retrieval_knn).

Computes, for full inputs hidden_states [1024, 256, 1024] f32 and
prototype_weight [512, 1024] f32:

    a      = mean(hidden_states, axis=1)                  # [B, D]
    logits = 2 a @ proto.T - ||a||^2 - ||proto||^2        # [B, 512]
    out    = logits.reshape(B, 64, 8).mean(axis=1)        # [B, 8]

Sharding: data-parallel over batch across 8 NeuronCores (128 batch rows
per core, prototype table replicated). The dominant cost is streaming the
128 MiB hidden_states shard from HBM.

DMA layout: strided partition reads (1 MiB partition stride) only reach
~190 GB/s/core on this part, while fully-linear reads reach ~350 GB/s.
Tiles are therefore loaded as flat contiguous [128, WPP] blocks. With
WPP words per partition, partition p of tile i holds WPP/1024 consecutive
t-rows; pooling reduces over t in up to two stages:
  stage 1 (only if WPP > 1024): DVE adds fold the in-partition t-rows;
  stage 2: a PE matmul with a sliding block-column mask (value 1/T)
  scatter-accumulates each batch's partitions into PSUM a[128b, 1024d].
At WPP=1024 the PE consumes raw tiles directly and the vector engine is
not involved in pooling at all.
"""

import os

os.environ.setdefault("JAX_PLATFORMS", "axon,cpu")

from contextlib import ExitStack

import numpy as np

B, T, D = 1024, 256, 1024
M_PROTO = 512
NUM_LABELS = 8
NUM_PROTOTYPES = 64
N_CORES = 8
BS = B // N_CORES  # 128 batch rows per core
P = 128            # SBUF partitions
WPP = 2048         # words per partition per DMA tile (tile = P*WPP*4 bytes)
HS_BUFS = 6

_cached = {}


def _build_program(reps=1, wpp=WPP, hs_bufs=HS_BUFS, act_pt2=False,
                   ttr_asq=False, stage1=True, split_dma=False):
    import concourse.mybir as mybir
    import concourse.tile as tile
    from concourse import bacc, masks

    f32 = mybir.dt.float32
    KD = D // P                      # 8 contraction chunks of 128 over D
    MG = M_PROTO // P                # 4 prototype groups of 128
    words_per_tile = P * wpp
    NT = (BS * T * D) // words_per_tile  # linear tiles per shard
    n_rows = wpp // D                # t-rows per partition (stage-1 depth)
    assert wpp % D == 0
    # batches per tile as a fraction: bpt_num/bpt_den
    bpt_num, bpt_den = words_per_tile, T * D
    n_cols = max(bpt_num // bpt_den, 1)   # mask columns per tile
    grp = P // n_cols if bpt_num >= bpt_den else P

    nc = bacc.Bacc("TRN2", target_bir_lowering=False, debug=False,
                   num_devices=N_CORES)
    hs = nc.dram_tensor("hidden_states", [BS, T, D], f32, kind="ExternalInput").ap()
    pw = nc.dram_tensor("prototype_weight", [M_PROTO, D], f32, kind="ExternalInput").ap()
    out = nc.dram_tensor("out", [BS, NUM_LABELS], f32, kind="ExternalOutput").ap()

    hs_flat = hs.rearrange("b t d -> (b t d)")

    with tile.TileContext(nc) as tc, ExitStack() as ctx:
        hs_pool = ctx.enter_context(tc.tile_pool(name="hs", bufs=hs_bufs))
        part_pool = ctx.enter_context(tc.tile_pool(name="part", bufs=3))
        work = ctx.enter_context(tc.tile_pool(name="work", bufs=1))
        psum_t = ctx.enter_context(tc.tile_pool(name="psum_t", bufs=2, space="PSUM"))
        psum_a = ctx.enter_context(tc.tile_pool(name="psum_a", bufs=1, space="PSUM"))

        state = {}

        def prep():
            ident = work.tile([P, P], f32, tag="ident", name="ident")
            masks.make_identity(nc, ident[:])
            ones_m1 = work.tile([P, 1], f32, tag="ones_m1", name="ones_m1")
            nc.gpsimd.memset(ones_m1[:], 1.0)
            ones_k1 = work.tile([1, P], f32, tag="ones_k1", name="ones_k1")
            nc.gpsimd.memset(ones_k1[:], 1.0)

            # Sliding mask for stage-2 pooling: zp[p, P + c] = 1/T iff
            # c == p // grp (c < n_cols). lhsT for tile i is
            # zp[:, P - s_i : 2P - s_i] with s_i = floor(i * bpt).
            zp = work.tile([P, 2 * P], f32, tag="zp", name="zp")
            nc.gpsimd.memset(zp[:], 0.0)
            for c in range(n_cols):
                nc.gpsimd.memset(zp[grp * c:grp * (c + 1), P + c:P + c + 1],
                                 1.0 / T)

            # protoT2[k] = 2 * proto.T d-chunk; sqT[k] = (2 proto.T)^2
            proto_sb = []
            for j in range(MG):
                pj = work.tile([P, D], f32, tag=f"proto{j}", name=f"proto{j}")
                nc.gpsimd.dma_start(pj[:], pw[j * P:(j + 1) * P, :])
                proto_sb.append(pj)

            protoT2 = [work.tile([P, M_PROTO], f32, tag=f"pT2_{k}", name=f"pT2_{k}")
                       for k in range(KD)]
            sqT = [work.tile([P, M_PROTO], f32, tag=f"sqT_{k}", name=f"sqT_{k}")
                   for k in range(KD)]
            for k in range(KD):
                for j in range(MG):
                    pt = psum_t.tile([P, P], f32, tag="tp", name="pt")
                    nc.tensor.transpose(pt[:], proto_sb[j][:, k * P:(k + 1) * P],
                                        ident[:])
                    if act_pt2:
                        nc.scalar.mul(protoT2[k][:, j * P:(j + 1) * P],
                                      pt[:], 2.0)
                    else:
                        nc.vector.tensor_scalar_mul(
                            protoT2[k][:, j * P:(j + 1) * P], pt[:], 2.0)
                # (2 protoT)^2 = 4 protoT^2; compensated below via -0.25 scale
                nc.vector.tensor_mul(sqT[k][:], protoT2[k][:], protoT2[k][:])

            # b_sq[m] as a [1, 512] row via ones-matmul over squared protoT
            bsq_ps = psum_a.tile([1, M_PROTO], f32, tag="bsq", name="bsq_ps")
            for k in range(KD):
                nc.tensor.matmul(bsq_ps[:], ones_m1[:], sqT[k][:],
                                 start=(k == 0), stop=(k == KD - 1))
            neg_bsq = work.tile([1, M_PROTO], f32, tag="neg_bsq", name="neg_bsq")
            nc.scalar.mul(neg_bsq[:], bsq_ps[:], -0.25)

            state.update(ident=ident, ones_k1=ones_k1, zp=zp, neg_bsq=neg_bsq,
                         protoT2=protoT2)

        def stream():
            ident = state["ident"]
            zp = state["zp"]
            protoT2 = state["protoT2"]

            # --- pooling: a[b, d] = (1/T) sum_t hs[b, t, d], in PSUM
            a_ps = psum_a.tile([P, D], f32, tag="a_ps", name="a_ps")
            dma_eng = [nc.sync, nc.scalar]
            for it in range(NT):
                tl = hs_pool.tile([P, wpp], f32, tag="hs", name="tl")
                src = hs_flat[it * words_per_tile:(it + 1) * words_per_tile]
                s2 = src.rearrange("(p w) -> p w", p=P)
                if split_dma:
                    # both HWDGE rings busy every tile: each ring moves a
                    # contiguous half (partition-split keeps linearity)
                    nc.sync.dma_start(tl[0:P // 2, :], s2[0:P // 2, :])
                    nc.scalar.dma_start(tl[P // 2:P, :], s2[P // 2:P, :])
                else:
                    dma_eng[it % 2].dma_start(tl[:], s2)
                s_i = (it * bpt_num) // bpt_den
                lhsT = zp[:, P - s_i:2 * P - s_i]
                if stage1 and n_rows > 1:
                    partial = part_pool.tile([P, D], f32, tag="part",
                                             name="partial")
                    nc.vector.tensor_add(partial[:], tl[:, 0:D], tl[:, D:2 * D])
                    for j in range(2, n_rows):
                        nc.vector.tensor_add(partial[:], partial[:],
                                             tl[:, j * D:(j + 1) * D])
                    for h in range(2):
                        nc.tensor.matmul(a_ps[:, h * 512:(h + 1) * 512], lhsT,
                                         partial[:, h * 512:(h + 1) * 512],
                                         start=(it == 0), stop=(it == NT - 1),
                                         skip_group_check=True)
                else:
                    # PE consumes raw t-rows directly; all rows of a tile
                    # share the same mask column (same batch coverage)
                    for r in range(n_rows):
                        for h in range(2):
                            nc.tensor.matmul(
                                a_ps[:, h * 512:(h + 1) * 512], lhsT,
                                tl[:, r * D + h * 512:r * D + (h + 1) * 512],
                                start=(it == 0 and r == 0),
                                stop=(it == NT - 1 and r == n_rows - 1),
                                skip_group_check=True)

            a_sb = work.tile([P, D], f32, tag="a", name="a_sb")
            nc.scalar.mul(a_sb[:], a_ps[:], 1.0)

            # a_sq[b] = sum_d a^2 as per-partition scalar [128, 1]
            sq_tmp = work.tile([P, D], f32, tag="sq_tmp", name="sq_tmp")
            asq = work.tile([P, 1], f32, tag="asq", name="asq")
            if ttr_asq:
                nc.vector.tensor_tensor_reduce(
                    out=sq_tmp[:], in0=a_sb[:], in1=a_sb[:], scale=1.0,
                    scalar=0.0, op0=mybir.AluOpType.mult,
                    op1=mybir.AluOpType.add, accum_out=asq[:])
            else:
                nc.vector.tensor_mul(sq_tmp[:], a_sb[:], a_sb[:])
                nc.vector.tensor_reduce(asq[:], sq_tmp[:],
                                        axis=mybir.AxisListType.X,
                                        op=mybir.AluOpType.add)

            # aT[k] = a.T d-chunk [128d, 128b]
            aTs = []
            for k in range(KD):
                pt = psum_t.tile([P, P], f32, tag="tp", name="pt")
                nc.tensor.transpose(pt[:], a_sb[:, k * P:(k + 1) * P], ident[:])
                aT = work.tile([P, P], f32, tag=f"aT{k}", name=f"aT{k}")
                nc.vector.tensor_copy(aT[:], pt[:])
                aTs.append(aT)

            # logits_pre[b, m] = 2 a@proto.T - b_sq in one PSUM bank
            lg_ps = psum_a.tile([P, M_PROTO], f32, tag="lg", name="lg_ps")
            for k in range(KD):
                nc.tensor.matmul(lg_ps[:], aTs[k][:], protoT2[k][:],
                                 start=(k == 0), stop=False)
            nc.tensor.matmul(lg_ps[:], state["ones_k1"][:], state["neg_bsq"][:],
                             start=False, stop=True)

            # subtract a_sq (per-partition scalar broadcast along free dim)
            lg_sb = work.tile([P, M_PROTO], f32, tag="lg_sb", name="lg_sb")
            nc.vector.tensor_scalar_sub(lg_sb[:], lg_ps[:], asq[:])

            # label mean: out[b, l] = mean_p logits_pre[b, p*8 + l]
            out_sb = work.tile([P, NUM_LABELS], f32, tag="out_sb", name="out_sb")
            lgv = lg_sb[:].rearrange("b (p l) -> b l p", l=NUM_LABELS)
            nc.vector.tensor_reduce(out_sb[:], lgv, axis=mybir.AxisListType.X,
                                    op=mybir.AluOpType.add)
            nc.scalar.mul(out_sb[:], out_sb[:], 1.0 / NUM_PROTOTYPES)
            nc.gpsimd.dma_start(out[:, :], out_sb[:])

        prep()
        if reps == 1:
            stream()
        else:
            hints = (mybir.EngineType.DVE, mybir.EngineType.PE,
                     mybir.EngineType.Activation, mybir.EngineType.SP,
                     mybir.EngineType.Pool)
            with tc.For_i(0, reps, 1, hint_engines=hints):
                stream()

    nc.compile()
    return nc


def _get_program(reps=1, **kw):
    key = (reps, tuple(sorted(kw.items())))
    if key not in _cached:
        _cached[key] = _build_program(reps, **kw)
    return _cached[key]


def _make_in_maps(hs, pw):
    return [
        {
            "hidden_states": np.ascontiguousarray(hs[i * BS:(i + 1) * BS]),
            "prototype_weight": pw,
        }
        for i in range(N_CORES)
    ]


def run(hidden_states, prototype_weight, trace=False, reps=1):
    """Run the SPMD kernel; returns (full_output, BassKernelResults)."""
    from concourse.bass_utils import run_bass_kernel_spmd

    hs = np.ascontiguousarray(np.asarray(hidden_states, dtype=np.float32))
    pw = np.ascontiguousarray(np.asarray(prototype_weight, dtype=np.float32))
    assert hs.shape == (B, T, D), hs.shape
    assert pw.shape == (M_PROTO, D), pw.shape

    nc = _get_program(reps)
    res = run_bass_kernel_spmd(nc, _make_in_maps(hs, pw),
                               core_ids=list(range(N_CORES)), trace=trace)
    full = np.concatenate([res.results[i]["out"] for i in range(N_CORES)], axis=0)
    return full, res


def kernel(hidden_states, prototype_weight):
    full, _ = run(hidden_states, prototype_weight, trace=False)
    return full
